# revision 47
# baseline (speedup 1.0000x reference)
"""Trainium2 Bass kernel for nn_ContrastiveLoss (ragged_sequence), final.

Math (see reference): a cross-attention t2i score matrix scores[i, c] over
B=64 images x B=64 captions, then a max-violation margin loss.

Sharding: caption slots are sharded 8-per-core across 8 NeuronCores with a
two-width ragged layout (short captions in WS-wide slots, the rest in
50-wide slots, chosen per call from s_l and compiled per layout); images
are replicated.  Each core computes its [64, 8] slot block of the score
matrix; the host un-permutes slots and runs the tiny margin reduction.

Design (162us baseline -> ~70us):
  * all matmuls bf16 with 128-column stationaries (FWL weight loads);
    packs of 3 images (108 partitions) processed in pairs so elementwise
    ops cover 2 packs per instruction; PSUM pair tiles at bank offsets.
  * software pipeline at pair granularity: sweep1 (DMA + 16-matmul
    A-burst + Prelu/Copy + square/word-norm) runs 3-4 steps ahead of
    sweep2 (an/E/H/E*A/E*H + r-reduction accumulations), with per-engine
    queue order arranged so the in-order queues never cross-block and the
    PE stays dense (HAM stays at K=8/8).
  * the only ACT functions anywhere are Prelu/Copy/Exp (one table set,
    loaded once); every rsqrt is a magic-seed + one-Newton-step DVE
    sequence (the 9x softmax temperature is folded into the step).
  * Z-cancellation: sim = ns/(cn*sqrt(ws)) -- the softmax denominator
    cancels, so no Z accumulator, no reciprocals.
  * engine split per pair: PE 22 matmuls, ACT Prelu+Copy+Exp,
    DVE square+reduce+newton+E*A+E*H, GPSIMD an.
"""

import sys

if "/opt/trn_rl_repo" not in sys.path:
    sys.path.insert(0, "/opt/trn_rl_repo")

import numpy as np

B, R, W, D = 64, 36, 50, 1024
NCORES = 8
CPC = B // NCORES          # caption slots per core = 8
PACK = 3                   # images per pack
NPACK = 22                 # ceil(64 / 3) -> 66 rows incl. 2 pad images
NPAIR = NPACK // 2         # 11
BP = NPACK * PACK          # 66
PPART = PACK * R           # 108 partitions per pack
KCH = D // 128             # 8 contraction chunks
SW = 128                   # stationary width: pad to 128 cols to enable FWL
IMC = KCH * SW             # 1024 im columns per pack (108 real + 20 pad)
PKC = IMC + 2 * SW         # 1280 combined pack columns (im | G | ones)
PB = 512                   # PSUM bank stride (fp32 elems) for pair tiles

# newton batches in pairs; groups of 2 give a seamless an-stream
# (newton(g) always lands >= 1 step before its first consumer)
PGROUPS = [[0, 1], [2, 3], [4, 5], [6, 7], [8, 9], [10]]

MARGIN = 0.2
LAM_SM = 9.0
LAM_LSE = 6.0

_PROGRAM_CACHE: dict = {}


def choose_layout(s_l: np.ndarray):
    """Two-width caption packing: NS short slots of width WS plus
    (8 - NS) long slots of width 50 per core.  Captions are assigned to
    slots sorted by length, dealt round-robin across cores.  Returns
    (ws, ns, caps) with caps[c][k] = original caption index in core c,
    slot k."""
    s_l = np.asarray(s_l).astype(np.int64)
    best = (CPC * W, W, 0)  # (ncw, ws, ns)
    for ws in range(4, W + 1, 2):
        ns = min(int((s_l <= ws).sum()) // NCORES, CPC)
        ncw = ns * ws + (CPC - ns) * W
        if ncw < best[0]:
            best = (ncw, ws, ns)
    _, ws, ns = best
    order = np.argsort(s_l, kind="stable")
    caps = np.empty((NCORES, CPC), np.int64)
    for k in range(CPC):
        for c in range(NCORES):
            if k < ns:
                caps[c, k] = order[k * NCORES + c]
            else:
                caps[c, k] = order[ns * NCORES + (k - ns) * NCORES + c]
    return ws, ns, caps


def build_program(ws: int, ns: int, debug: bool = False):
    """Build the per-core Bass/Tile program (identical on all 8 cores).
    ws/ns define the two-width caption slot layout."""
    import concourse.bacc as bacc
    import concourse.mybir as mybir
    import concourse.tile as tile

    f32 = mybir.dt.float32
    bf16 = mybir.dt.bfloat16
    i32 = mybir.dt.int32
    AF = mybir.ActivationFunctionType
    ALU = mybir.AluOpType
    AX = mybir.AxisListType

    nl = CPC - ns              # long slots per core
    SLB = ns * ws              # short-region column count
    NCW = SLB + nl * W         # per-core (c, w) columns

    nc = bacc.Bacc("TRN2", target_bir_lowering=False, debug=debug)

    pk_d = nc.dram_tensor("pk", [NPAIR, 128, 2 * PKC], bf16, kind="ExternalInput")
    s_d = nc.dram_tensor("s_packed", [128, KCH * NCW], bf16, kind="ExternalInput")
    nlc_d = nc.dram_tensor("neglncn", [BP, NCW], f32, kind="ExternalInput")
    pc_d = nc.dram_tensor("padc66", [BP, CPC], f32, kind="ExternalInput")
    out_d = nc.dram_tensor("scores8", [B, CPC], f32, kind="ExternalOutput")

    group_of = {}
    for g, pairs in enumerate(PGROUPS):
        for j in pairs:
            group_of[j] = g
    NMAX = 16 * max(len(gg) for gg in PGROUPS)

    MAXLIVE = 8  # al2/asb2/pk2 pair lifetime (sweep1 .. sweep2)

    with tile.TileContext(nc) as tc:
        with (
            tc.tile_pool(name="const", bufs=1) as cpool,
            tc.tile_pool(name="pk", bufs=MAXLIVE) as pkpool,
            tc.tile_pool(name="ala", bufs=MAXLIVE) as alpool,
            tc.tile_pool(name="sqp", bufs=2) as sqpool,
            tc.tile_pool(name="anp", bufs=3) as anpool,
            tc.tile_pool(name="ep", bufs=3) as epool,
            tc.tile_pool(name="eaeh", bufs=2) as eapool,
            tc.tile_pool(name="nwt", bufs=2) as nwt,
            tc.tile_pool(name="ph2", bufs=2) as ph2,
            tc.tile_pool(name="psA", bufs=2, space="PSUM") as psA,
            tc.tile_pool(name="psH", bufs=1, space="PSUM") as psH,
            tc.tile_pool(name="psacc", bufs=1, space="PSUM") as psacc,
        ):
            # tiles for the shared inputs; DMAs are emitted inside the
            # pipeline (pk(0) first so the first A-burst starts ASAP,
            # phase-2-only inputs deferred)
            s_sb = cpool.tile([128, KCH * NCW], bf16)
            nlc_sb = cpool.tile([BP, NCW], f32)
            pc_sb = cpool.tile([BP, CPC], f32)

            # magic constant for the rsqrt newton seed
            magic = cpool.tile([PPART, 1], i32)
            nc.vector.memset(magic[:], 0x5F3759DF)

            # persistent per-(b,r) x caption tiles for s2 / 9*rsqrt(s2)
            s2_all = cpool.tile([PPART, NPACK * CPC], f32)
            rn9_all = cpool.tile([PPART, NPACK * CPC], f32)

            # persistent PSUM accumulators (fp32): ns = sum_r E*A,
            # ws = sum_r E*H, accumulated over all 22 packs.  Partition dim
            # is 128 (ones stationaries are padded to 128 cols for FWL);
            # rows 66..127 are junk from the zero padding.
            nz_acc = psacc.tile([128, NCW], f32)
            wz_acc = psacc.tile([128, NCW], f32)

            al_t: dict = {}
            asb_t: dict = {}
            pk_t: dict = {}
            e_t: dict = {}

            def pair_view(t):
                """[108, 2, 400] view of a [128, 2*PB] PSUM pair tile."""
                return t[0:PPART].rearrange("p (u x) -> p u x", u=2)[:, :, 0:NCW]

            a_ps_t: dict = {}

            def sweep1_mm(j):
                pk_sb = pkpool.tile([128, 2 * PKC], bf16, tag="pk")
                if j == 0:
                    # s chunks 0-1 land first so the first A-burst can
                    # start; later parts are emitted between the matmuls
                    # that consume them and stream under the burst
                    nc.sync.dma_start(s_sb[:, :2 * NCW], s_d[:, :2 * NCW])
                nc.sync.dma_start(pk_sb[:], pk_d[j])
                pk_t[j] = pk_sb
                if j == 2:
                    nc.sync.dma_start(nlc_sb[:], nlc_d[:])
                    nc.sync.dma_start(pc_sb[:], pc_d[:])

                # A[108, 2x400] = sum_k im_pack_k^T @ s_k  (bf16, 128-col
                # stationaries -> FWL; output rows 108..127 are zero junk)
                a_ps = psA.tile([128, 2 * PB], f32)
                if j == 0:
                    for k in range(KCH):
                        if k == 2:
                            nc.sync.dma_start(s_sb[:, 2 * NCW:5 * NCW],
                                              s_d[:, 2 * NCW:5 * NCW])
                        if k == 5:
                            nc.sync.dma_start(s_sb[:, 5 * NCW:],
                                              s_d[:, 5 * NCW:])
                        for u in range(2):
                            nc.tensor.matmul(
                                a_ps[:, u * PB:u * PB + NCW],
                                pk_sb[:, u * PKC + k * SW:u * PKC + (k + 1) * SW],
                                s_sb[:, k * NCW:(k + 1) * NCW],
                                start=(k == 0),
                                stop=(k == KCH - 1),
                            )
                else:
                    for u in range(2):
                        for k in range(KCH):
                            nc.tensor.matmul(
                                a_ps[:, u * PB:u * PB + NCW],
                                pk_sb[:, u * PKC + k * SW:u * PKC + (k + 1) * SW],
                                s_sb[:, k * NCW:(k + 1) * NCW],
                                start=(k == 0),
                                stop=(k == KCH - 1),
                            )
                a_ps_t[j] = a_ps

            def sweep1_post(j):
                a_ps = a_ps_t.pop(j)
                # al = leaky_relu(A, 0.1), a_sb = raw A -> dense SBUF bf16
                al = alpool.tile([PPART, 2 * NCW], bf16, tag="al")
                nc.scalar.activation(
                    al[:].rearrange("p (u x) -> p u x", u=2),
                    pair_view(a_ps), AF.Prelu, alpha=0.1,
                )
                a_sb = alpool.tile([PPART, 2 * NCW], bf16, tag="asb")
                nc.scalar.activation(
                    a_sb[:].rearrange("p (u x) -> p u x", u=2),
                    pair_view(a_ps), AF.Copy,
                )
                al_t[j], asb_t[j] = al, a_sb

                # s2[108, 16] = sum_w al^2  (square + reduce on DVE;
                # one reduce per slot-width region, covering both packs)
                sq = sqpool.tile([PPART, 2 * NCW], bf16, tag="sq")
                nc.vector.tensor_mul(sq[:], al[:], al[:])
                sqr = sq[:].rearrange("p (u x) -> p u x", u=2)
                s2r = s2_all[:, j * 2 * CPC:(j + 1) * 2 * CPC].rearrange(
                    "p (u c) -> p u c", u=2)
                if ns > 0:
                    nc.vector.tensor_reduce(
                        s2r[:, :, 0:ns],
                        sqr[:, :, 0:SLB].rearrange("p u (c w) -> p u c w",
                                                   c=ns),
                        AX.X, ALU.add,
                    )
                if nl > 0:
                    nc.vector.tensor_reduce(
                        s2r[:, :, ns:CPC],
                        sqr[:, :, SLB:NCW].rearrange("p u (c w) -> p u c w",
                                                     c=nl),
                        AX.X, ALU.add,
                    )

            def newton(g):
                """rn9 = 9 / sqrt(s2) for the group's captions: magic seed
                + 2 fused Newton steps.  The max clamp keeps all-zero
                columns (pad images) finite through the iteration."""
                pairs = PGROUPS[g]
                lo, hi = pairs[0] * 2 * CPC, (pairs[-1] + 1) * 2 * CPC
                n = hi - lo

                def tl(tag, dt=f32):
                    t = nwt.tile([PPART, NMAX], dt, tag=tag, name=f"nwt_{tag}")
                    return t[:, :n]

                x = tl("x")
                nc.vector.tensor_scalar_max(x, s2_all[:, lo:hi], 1e-20)
                t1 = tl("t1", i32)
                nc.vector.tensor_scalar(
                    t1, x.bitcast(i32), 1, None, op0=ALU.logical_shift_right
                )
                y0 = tl("y0")
                nc.vector.tensor_tensor(
                    y0.bitcast(i32),
                    magic[:].broadcast_to([PPART, n]),
                    t1,
                    op=ALU.subtract,
                )
                # one Newton step with the 9x softmax temperature folded in:
                # rn9 = (13.5 + (-4.5 * y0^2) * x) * y0 = 9 * rsqrt(x)
                # (seed err ~3.4% -> ~0.2% after one step; loss impact
                # measured at ~4e-5 relative, far inside tolerance)
                a1 = tl("a1")
                nc.vector.tensor_mul(a1, y0, y0)
                nc.vector.scalar_tensor_tensor(a1, a1, -4.5, x, ALU.mult,
                                               ALU.mult)
                nc.vector.scalar_tensor_tensor(rn9_all[:, lo:hi], a1, 13.5,
                                               y0, ALU.add, ALU.mult)

            an_t: dict = {}

            def sweep2_an(j):
                # an = al * (9 * rsqrt(s2)) broadcast over words  (GPSIMD,
                # one op per slot-width region covering both packs)
                al = al_t.pop(j)
                an = anpool.tile([PPART, 2 * NCW], f32, tag="an")
                anr = an[:].rearrange("p (u x) -> p u x", u=2)
                alr = al[:].rearrange("p (u x) -> p u x", u=2)
                rn = rn9_all[:, j * 2 * CPC:(j + 1) * 2 * CPC].rearrange(
                    "p (u c) -> p u c", u=2)
                if ns > 0:
                    nc.gpsimd.tensor_mul(
                        anr[:, :, 0:SLB].rearrange("p u (c w) -> p u c w",
                                                   c=ns),
                        alr[:, :, 0:SLB].rearrange("p u (c w) -> p u c w",
                                                   c=ns),
                        rn[:, :, 0:ns].broadcast_to([PPART, 2, ns, ws]),
                    )
                if nl > 0:
                    nc.gpsimd.tensor_mul(
                        anr[:, :, SLB:NCW].rearrange("p u (c w) -> p u c w",
                                                     c=nl),
                        alr[:, :, SLB:NCW].rearrange("p u (c w) -> p u c w",
                                                     c=nl),
                        rn[:, :, ns:CPC].broadcast_to([PPART, 2, nl, W]),
                    )
                an_t[j] = an

            def sweep2_exp(j):
                an = an_t.pop(j)
                e = epool.tile([PPART, 2 * NCW], bf16, tag="e")
                nc.scalar.activation(e[:], an[:], AF.Exp)
                e_t[j] = e

            def sweep2_rest(j):
                a_sb, pk_sb, e = asb_t.pop(j), pk_t.pop(j), e_t.pop(j)

                # H = Gbd @ E per pack; ea/eh pair-wide on DVE; the two
                # r-reduction accumulations (G/ones padded to 128 for FWL)
                h_ps = psH.tile([128, 2 * PB], f32)
                for u in range(2):
                    nc.tensor.matmul(
                        h_ps[:, u * PB:u * PB + NCW],
                        pk_sb[:PPART, u * PKC + IMC:u * PKC + IMC + SW],
                        e[:, u * NCW:(u + 1) * NCW],
                        start=True, stop=True,
                    )
                ea = eapool.tile([PPART, 2 * NCW], bf16, tag="ea")
                # in the pipeline drain there are no A-bursts and DVE gates
                # the dribble; hand E*A to the otherwise-idle GPSIMD there
                eng = nc.gpsimd if j >= NPAIR - 3 else nc.vector
                eng.tensor_mul(ea[:], e[:], a_sb[:])
                for u in range(2):
                    p = 2 * j + u
                    nc.tensor.matmul(
                        nz_acc[:],
                        pk_sb[:PPART, u * PKC + IMC + SW:u * PKC + IMC + 2 * SW],
                        ea[:, u * NCW:(u + 1) * NCW],
                        start=(p == 0), stop=(p == NPACK - 1),
                    )
                eh = eapool.tile([PPART, 2 * NCW], bf16, tag="eh")
                nc.vector.tensor_mul(
                    eh[:].rearrange("p (u x) -> p u x", u=2),
                    e[:].rearrange("p (u x) -> p u x", u=2),
                    pair_view(h_ps),
                )
                for u in range(2):
                    p = 2 * j + u
                    nc.tensor.matmul(
                        wz_acc[:],
                        pk_sb[:PPART, u * PKC + IMC + SW:u * PKC + IMC + 2 * SW],
                        eh[:, u * NCW:(u + 1) * NCW],
                        start=(p == 0), stop=(p == NPACK - 1),
                    )

            # ---- software-pipelined emission over pair steps ----
            # Per step t the per-engine queue order is arranged so nothing
            # blocks in-order: GPSIMD computes an(j+1) a step early, ACT
            # runs Exp(j) before sweep1(t)'s Prelu/Copy/Square, and the PE
            # queue holds [A-burst(t), H/acc(j)] so e(j) is long done when
            # PE reaches H(j).
            newton_done = [False] * len(PGROUPS)
            an_step: dict = {}
            n_an = 0     # next pair needing an
            n_rest = 0   # next pair needing e/H/ea/eh/accs
            t = 0
            while n_rest < NPAIR:
                if (n_an < NPAIR and n_an < t
                        and newton_done[group_of[n_an]]):
                    sweep2_an(n_an)
                    an_step[n_an] = t
                    n_an += 1
                lag = 2 if t < NPAIR else 0
                do_rest = (n_rest < n_an
                           and an_step[n_rest] <= t - lag)
                if do_rest:
                    sweep2_exp(n_rest)
                if t < NPAIR:
                    sweep1_mm(t)
                if do_rest:
                    sweep2_rest(n_rest)
                    n_rest += 1
                if t < NPAIR:
                    sweep1_post(t)
                    g = group_of[t]
                    if t == PGROUPS[g][-1]:
                        newton(g)
                        newton_done[g] = True
                t += 1

            # ---- phase 2: sim = ns * rsqrt(ws) / cn, LSE over words ----
            # rsqrt on DVE (magic + 1 Newton step) so no ACT table switch
            # is needed: the exp table from the main loop stays loaded.
            wsm = ph2.tile([BP, NCW], f32, tag="wsm")
            nc.vector.tensor_scalar_max(wsm[:], wz_acc[0:BP, :], 1e-20)
            pt1 = ph2.tile([BP, NCW], i32, tag="pt1")
            nc.vector.tensor_scalar(
                pt1[:], wsm[:].bitcast(i32), 1, None,
                op0=ALU.logical_shift_right
            )
            py0 = ph2.tile([BP, NCW], f32, tag="py0")
            nc.vector.tensor_tensor(
                py0[:].bitcast(i32),
                magic[0:BP].broadcast_to([BP, NCW]),
                pt1[:],
                op=ALU.subtract,
            )
            pa1 = ph2.tile([BP, NCW], f32, tag="pa1")
            nc.vector.tensor_mul(pa1[:], py0[:], py0[:])
            nc.vector.scalar_tensor_tensor(pa1[:], pa1[:], -0.5, wsm[:],
                                           ALU.mult, ALU.mult)
            rn = ph2.tile([BP, NCW], f32, tag="rn")
            nc.vector.scalar_tensor_tensor(rn[:], pa1[:], 1.5, py0[:],
                                           ALU.add, ALU.mult)
            # sim = (rsqrt(ws) * (1/cn)) * ns
            qq = ph2.tile([BP, NCW], f32, tag="qq")
            nc.vector.tensor_mul(qq[:], rn[:], nlc_sb[:])
            sim = ph2.tile([BP, NCW], f32, tag="sim")
            nc.vector.tensor_mul(sim[:], qq[:], nz_acc[0:BP, :])
            ee = ph2.tile([BP, NCW], f32, tag="ee")
            nc.scalar.activation(ee[:], sim[:], AF.Exp, scale=LAM_LSE)
            rowz = ph2.tile([BP, CPC], f32, tag="rowz")
            eer = ee[:]
            if ns > 0:
                nc.vector.tensor_reduce(
                    rowz[:, 0:ns],
                    eer[:, 0:SLB].rearrange("p (c w) -> p c w", c=ns),
                    AX.X, ALU.add,
                )
            if nl > 0:
                nc.vector.tensor_reduce(
                    rowz[:, ns:CPC],
                    eer[:, SLB:NCW].rearrange("p (c w) -> p c w", c=nl),
                    AX.X, ALU.add,
                )
            rowc = ph2.tile([BP, CPC], f32, tag="rowc")
            nc.vector.tensor_sub(rowc[:], rowz[:], pc_sb[:])

            # host finishes with ln(rowc)/6
            nc.sync.dma_start(out_d[:], rowc[0:B, :])

    nc.compile()
    return nc


def prepare_inputs(im: np.ndarray, s: np.ndarray, s_l: np.ndarray):
    """Host-side input marshalling: shard captions, transpose to d-major,
    pack images into 3-image/108-partition packs (paired, one DMA buffer
    per 2 packs), block-diagonal Gram + ones stationaries, -ln(caption
    norms) and pad counts."""
    import ml_dtypes

    bf16 = ml_dtypes.bfloat16
    im = np.ascontiguousarray(np.asarray(im, np.float32))
    s = np.ascontiguousarray(np.asarray(s, np.float32))
    s_l = np.asarray(s_l).astype(np.int64)

    # zero out padded words so A columns for padded (c, w) are exactly 0
    wmask = (np.arange(W)[None, :] < s_l[:, None])          # [64, 50]
    s_z = s * wmask[:, :, None].astype(np.float32)

    # im packs: [22, 128, 8*128], each 128-col chunk = 108 real + 20 zero
    imf = im.transpose(2, 0, 1).reshape(D, B * R)            # [1024, 2304]
    imf66 = np.zeros((D, BP * R), np.float32)
    imf66[:, : B * R] = imf
    im108 = (
        imf66.reshape(KCH, 128, NPACK, PPART)
        .transpose(2, 1, 0, 3)
        .reshape(NPACK, 128, KCH, PPART)
    )
    im_packed = np.zeros((NPACK, 128, KCH, SW), np.float32)
    im_packed[:, :, :, :PPART] = im108
    im_packed = im_packed.reshape(NPACK, 128, IMC)

    # Gram matrices, block-diagonal per pack: [22, 108, 108]
    G = np.matmul(im, im.transpose(0, 2, 1))                 # [64, 36, 36] f32
    gbd = np.zeros((NPACK, PPART, PPART), np.float32)
    for jj in range(PACK):
        for p in range(NPACK):
            b = PACK * p + jj
            if b < B:
                gbd[p, R * jj: R * (jj + 1), R * jj: R * (jj + 1)] = G[b]

    # ones_p stationaries: [22, 108, 66], 3 block-ones columns at 3p
    ones_p = np.zeros((NPACK, PPART, BP), np.float32)
    for p in range(NPACK):
        for jj in range(PACK):
            ones_p[p, R * jj: R * (jj + 1), PACK * p + jj] = 1.0

    # fused per-pack buffer [22, 128, 1024 im | 128 G | 128 ones], paired
    pkb = np.zeros((NPACK, 128, PKC), np.float32)
    pkb[:, :, :IMC] = im_packed
    pkb[:, :PPART, IMC:IMC + PPART] = gbd
    pkb[:, :PPART, IMC + SW:IMC + SW + BP] = ones_p
    pk = np.ascontiguousarray(
        pkb.reshape(NPAIR, 2, 128, PKC).transpose(0, 2, 1, 3)
        .reshape(NPAIR, 128, 2 * PKC).astype(bf16)
    )

    # 1/(caption word norms); padded words -> 0 (their ns column is 0,
    # so sim = 0 and each contributes exp(0)=1 to the row sum)
    cn = np.sqrt((s_z * s_z).sum(axis=2))                    # [64, 50]
    nlc = np.where(cn > 0, 1.0 / np.maximum(cn, 1e-30), 0.0).astype(
        np.float32
    )
    padc = (W - s_l).astype(np.float32)                      # [64]

    # slot widths per the two-width layout
    ws, ns, caps = choose_layout(s_l)
    widths = [ws] * ns + [W] * (CPC - ns)
    ncw = sum(widths)

    in_maps = []
    for c in range(NCORES):
        cc = caps[c]                                          # slot -> caption
        s_cols = np.concatenate(
            [s_z[cc[k], :widths[k], :] for k in range(CPC)], axis=0
        )                                                     # [ncw, 1024]
        sT = s_cols.T                                         # [1024, ncw]
        s_packed = np.ascontiguousarray(
            sT.reshape(KCH, 128, ncw).transpose(1, 0, 2).reshape(128, KCH * ncw)
            .astype(bf16)
        )
        nlc_c = np.concatenate([nlc[cc[k], :widths[k]] for k in range(CPC)])
        padc_c = np.array(
            [widths[k] - s_l[cc[k]] for k in range(CPC)], np.float32
        )
        nlc66 = np.broadcast_to(nlc_c.reshape(1, ncw), (BP, ncw))
        padc66 = np.broadcast_to(padc_c.reshape(1, CPC), (BP, CPC))
        in_maps.append(
            {
                "pk": pk,
                "s_packed": s_packed,
                "neglncn": np.ascontiguousarray(nlc66, dtype=np.float32),
                "padc66": np.ascontiguousarray(padc66, dtype=np.float32),
            }
        )
    return in_maps


def scores_from_results(res, s_l) -> np.ndarray:
    """res: list of per-core result dicts -> full [64, 64] score matrix,
    un-permuting the two-width slot assignment."""
    _, _, caps = choose_layout(s_l)
    scores = np.empty((B, B), np.float32)
    for c in range(NCORES):
        rowc = np.asarray(res[c]["scores8"], np.float32)      # [64, 8]
        sc = np.log(np.maximum(rowc, 1e-30)) / LAM_LSE
        for k in range(CPC):
            scores[:, caps[c, k]] = sc[:, k]
    return scores


def margin_loss(scores: np.ndarray) -> np.float32:
    scores = scores.astype(np.float32)
    diag = np.diag(scores).copy()
    cost_s = np.maximum(MARGIN + scores - diag[:, None], 0.0)
    cost_im = np.maximum(MARGIN + scores - diag[None, :], 0.0)
    np.fill_diagonal(cost_s, 0.0)
    np.fill_diagonal(cost_im, 0.0)
    return np.float32(cost_s.max(axis=1).sum() + cost_im.max(axis=0).sum())


def kernel(im: np.ndarray, s: np.ndarray, s_l: np.ndarray) -> np.ndarray:
    from concourse.bass_utils import run_bass_kernel_spmd

    ws, ns, _ = choose_layout(s_l)
    key = (ws, ns)
    if key not in _PROGRAM_CACHE:
        _PROGRAM_CACHE[key] = build_program(ws, ns)
    nc = _PROGRAM_CACHE[key]

    in_maps = prepare_inputs(im, s, s_l)
    res = run_bass_kernel_spmd(nc, in_maps, list(range(NCORES))).results
    return margin_loss(scores_from_results(res, s_l))


# revision 49
# speedup vs baseline: 1.0267x; 1.0267x over previous
"""Trainium2 Bass kernel for nn_ContrastiveLoss (ragged_sequence), final.

Math (see reference): a cross-attention t2i score matrix scores[i, c] over
B=64 images x B=64 captions, then a max-violation margin loss.

Sharding: caption slots are sharded 8-per-core across 8 NeuronCores with a
two-width ragged layout (short captions in WS-wide slots, the rest in
50-wide slots, chosen per call from s_l and compiled per layout); images
are replicated.  Each core computes its [64, 8] slot block of the score
matrix; the host un-permutes slots and runs the tiny margin reduction.

Design (162us baseline -> ~70us):
  * all matmuls bf16 with 128-column stationaries (FWL weight loads);
    packs of 3 images (108 partitions) processed in pairs so elementwise
    ops cover 2 packs per instruction; PSUM pair tiles at bank offsets.
  * software pipeline at pair granularity: sweep1 (DMA + 16-matmul
    A-burst + Prelu/Copy + square/word-norm) runs 3-4 steps ahead of
    sweep2 (an/E/H/E*A/E*H + r-reduction accumulations), with per-engine
    queue order arranged so the in-order queues never cross-block and the
    PE stays dense (HAM stays at K=8/8).
  * the only ACT functions anywhere are Prelu/Copy/Exp (one table set,
    loaded once); every rsqrt is a magic-seed + one-Newton-step DVE
    sequence (the 9x softmax temperature is folded into the step).
  * Z-cancellation: sim = ns/(cn*sqrt(ws)) -- the softmax denominator
    cancels, so no Z accumulator, no reciprocals.
  * engine split per pair: PE 22 matmuls, ACT Prelu+Copy+Exp,
    DVE square+reduce+newton+E*A+E*H, GPSIMD an.
"""

import sys

if "/opt/trn_rl_repo" not in sys.path:
    sys.path.insert(0, "/opt/trn_rl_repo")

import numpy as np

B, R, W, D = 64, 36, 50, 1024
NCORES = 8
CPC = B // NCORES          # caption slots per core = 8
PACK = 3                   # images per pack
NPACK = 22                 # ceil(64 / 3) -> 66 rows incl. 2 pad images
NPAIR = NPACK // 2         # 11
BP = NPACK * PACK          # 66
PPART = PACK * R           # 108 partitions per pack
KCH = D // 128             # 8 contraction chunks
SW = 128                   # stationary width: pad to 128 cols to enable FWL
IMC = KCH * SW             # 1024 im columns per pack (108 real + 20 pad)
PKC = IMC + 2 * SW         # 1280 combined pack columns (im | G | ones)
PB = 512                   # PSUM bank stride (fp32 elems) for pair tiles

# newton batches in pairs; groups of 2 give a seamless an-stream
# (newton(g) always lands >= 1 step before its first consumer)
PGROUPS = [[0, 1], [2, 3], [4, 5], [6, 7], [8, 9], [10]]

MARGIN = 0.2
LAM_SM = 9.0
LAM_LSE = 6.0

_PROGRAM_CACHE: dict = {}


def choose_layout(s_l: np.ndarray):
    """Two-width caption packing: NS short slots of width WS plus
    (8 - NS) long slots of width 50 per core.  Captions are assigned to
    slots sorted by length, dealt round-robin across cores.  Returns
    (ws, ns, caps) with caps[c][k] = original caption index in core c,
    slot k."""
    s_l = np.asarray(s_l).astype(np.int64)
    best = (CPC * W, W, 0)  # (ncw, ws, ns)
    for ws in range(4, W + 1, 2):
        ns = min(int((s_l <= ws).sum()) // NCORES, CPC)
        ncw = ns * ws + (CPC - ns) * W
        if ncw < best[0]:
            best = (ncw, ws, ns)
    _, ws, ns = best
    order = np.argsort(s_l, kind="stable")
    caps = np.empty((NCORES, CPC), np.int64)
    for k in range(CPC):
        for c in range(NCORES):
            if k < ns:
                caps[c, k] = order[k * NCORES + c]
            else:
                caps[c, k] = order[ns * NCORES + (k - ns) * NCORES + c]
    return ws, ns, caps


def build_program(ws: int, ns: int, debug: bool = False):
    """Build the per-core Bass/Tile program (identical on all 8 cores).
    ws/ns define the two-width caption slot layout."""
    import concourse.bacc as bacc
    import concourse.mybir as mybir
    import concourse.tile as tile

    f32 = mybir.dt.float32
    bf16 = mybir.dt.bfloat16
    i32 = mybir.dt.int32
    AF = mybir.ActivationFunctionType
    ALU = mybir.AluOpType
    AX = mybir.AxisListType

    nl = CPC - ns              # long slots per core
    SLB = ns * ws              # short-region column count
    NCW = SLB + nl * W         # per-core (c, w) columns

    nc = bacc.Bacc("TRN2", target_bir_lowering=False, debug=debug)

    pk_d = nc.dram_tensor("pk", [NPAIR, 128, 2 * PKC], bf16, kind="ExternalInput")
    s_d = nc.dram_tensor("s_packed", [128, KCH * NCW], bf16, kind="ExternalInput")
    nlc_d = nc.dram_tensor("neglncn", [BP, NCW], f32, kind="ExternalInput")
    pc_d = nc.dram_tensor("padc66", [BP, CPC], f32, kind="ExternalInput")
    out_d = nc.dram_tensor("scores8", [B, CPC], f32, kind="ExternalOutput")

    group_of = {}
    for g, pairs in enumerate(PGROUPS):
        for j in pairs:
            group_of[j] = g
    NMAX = 16 * max(len(gg) for gg in PGROUPS)

    MAXLIVE = 8  # al2/asb2/pk2 pair lifetime (sweep1 .. sweep2)

    with tile.TileContext(nc) as tc:
        with (
            tc.tile_pool(name="const", bufs=1) as cpool,
            tc.tile_pool(name="pk", bufs=MAXLIVE) as pkpool,
            tc.tile_pool(name="ala", bufs=MAXLIVE) as alpool,
            tc.tile_pool(name="sqp", bufs=3) as sqpool,
            tc.tile_pool(name="anp", bufs=3) as anpool,
            tc.tile_pool(name="ep", bufs=4) as epool,
            tc.tile_pool(name="eaeh", bufs=4) as eapool,
            tc.tile_pool(name="nwt", bufs=2) as nwt,
            tc.tile_pool(name="ph2", bufs=2) as ph2,
            tc.tile_pool(name="psA", bufs=2, space="PSUM") as psA,
            tc.tile_pool(name="psH", bufs=1, space="PSUM") as psH,
            tc.tile_pool(name="psacc", bufs=1, space="PSUM") as psacc,
        ):
            # tiles for the shared inputs; DMAs are emitted inside the
            # pipeline (pk(0) first so the first A-burst starts ASAP,
            # phase-2-only inputs deferred)
            s_sb = cpool.tile([128, KCH * NCW], bf16)
            nlc_sb = cpool.tile([BP, NCW], f32)
            pc_sb = cpool.tile([BP, CPC], f32)

            # magic constant for the rsqrt newton seed
            magic = cpool.tile([PPART, 1], i32)
            nc.vector.memset(magic[:], 0x5F3759DF)

            # persistent per-(b,r) x caption tiles for s2 / 9*rsqrt(s2)
            s2_all = cpool.tile([PPART, NPACK * CPC], f32)
            rn9_all = cpool.tile([PPART, NPACK * CPC], f32)

            # persistent PSUM accumulators (fp32): ns = sum_r E*A,
            # ws = sum_r E*H, accumulated over all 22 packs.  Partition dim
            # is 128 (ones stationaries are padded to 128 cols for FWL);
            # rows 66..127 are junk from the zero padding.
            nz_acc = psacc.tile([128, NCW], f32)
            wz_acc = psacc.tile([128, NCW], f32)

            al_t: dict = {}
            asb_t: dict = {}
            pk_t: dict = {}
            e_t: dict = {}

            def pair_view(t):
                """[108, 2, 400] view of a [128, 2*PB] PSUM pair tile."""
                return t[0:PPART].rearrange("p (u x) -> p u x", u=2)[:, :, 0:NCW]

            a_ps_t: dict = {}

            def sweep1_mm(j):
                pk_sb = pkpool.tile([128, 2 * PKC], bf16, tag="pk")
                if j == 0:
                    # s chunks 0-1 land first so the first A-burst can
                    # start; later parts are emitted between the matmuls
                    # that consume them and stream under the burst
                    nc.sync.dma_start(s_sb[:, :2 * NCW], s_d[:, :2 * NCW])
                nc.sync.dma_start(pk_sb[:], pk_d[j])
                pk_t[j] = pk_sb
                if j == 2:
                    nc.sync.dma_start(nlc_sb[:], nlc_d[:])
                    nc.sync.dma_start(pc_sb[:], pc_d[:])

                # A[108, 2x400] = sum_k im_pack_k^T @ s_k  (bf16, 128-col
                # stationaries -> FWL; output rows 108..127 are zero junk)
                a_ps = psA.tile([128, 2 * PB], f32)
                if j == 0:
                    for k in range(KCH):
                        if k == 2:
                            nc.sync.dma_start(s_sb[:, 2 * NCW:5 * NCW],
                                              s_d[:, 2 * NCW:5 * NCW])
                        if k == 5:
                            nc.sync.dma_start(s_sb[:, 5 * NCW:],
                                              s_d[:, 5 * NCW:])
                        for u in range(2):
                            nc.tensor.matmul(
                                a_ps[:, u * PB:u * PB + NCW],
                                pk_sb[:, u * PKC + k * SW:u * PKC + (k + 1) * SW],
                                s_sb[:, k * NCW:(k + 1) * NCW],
                                start=(k == 0),
                                stop=(k == KCH - 1),
                            )
                else:
                    for u in range(2):
                        for k in range(KCH):
                            nc.tensor.matmul(
                                a_ps[:, u * PB:u * PB + NCW],
                                pk_sb[:, u * PKC + k * SW:u * PKC + (k + 1) * SW],
                                s_sb[:, k * NCW:(k + 1) * NCW],
                                start=(k == 0),
                                stop=(k == KCH - 1),
                            )
                a_ps_t[j] = a_ps

            def sweep1_post(j):
                a_ps = a_ps_t.pop(j)
                # al = leaky_relu(A, 0.1), a_sb = raw A -> dense SBUF bf16
                al = alpool.tile([PPART, 2 * NCW], bf16, tag="al")
                nc.scalar.activation(
                    al[:].rearrange("p (u x) -> p u x", u=2),
                    pair_view(a_ps), AF.Prelu, alpha=0.1,
                )
                a_sb = alpool.tile([PPART, 2 * NCW], bf16, tag="asb")
                nc.scalar.activation(
                    a_sb[:].rearrange("p (u x) -> p u x", u=2),
                    pair_view(a_ps), AF.Copy,
                )
                al_t[j], asb_t[j] = al, a_sb

                # s2[108, 16] = sum_w al^2  (square + reduce on DVE;
                # one reduce per slot-width region, covering both packs)
                sq = sqpool.tile([PPART, 2 * NCW], bf16, tag="sq")
                nc.vector.tensor_mul(sq[:], al[:], al[:])
                sqr = sq[:].rearrange("p (u x) -> p u x", u=2)
                s2r = s2_all[:, j * 2 * CPC:(j + 1) * 2 * CPC].rearrange(
                    "p (u c) -> p u c", u=2)
                if ns > 0:
                    nc.vector.tensor_reduce(
                        s2r[:, :, 0:ns],
                        sqr[:, :, 0:SLB].rearrange("p u (c w) -> p u c w",
                                                   c=ns),
                        AX.X, ALU.add,
                    )
                if nl > 0:
                    nc.vector.tensor_reduce(
                        s2r[:, :, ns:CPC],
                        sqr[:, :, SLB:NCW].rearrange("p u (c w) -> p u c w",
                                                     c=nl),
                        AX.X, ALU.add,
                    )

            def newton(g):
                """rn9 = 9 / sqrt(s2) for the group's captions: magic seed
                + 2 fused Newton steps.  The max clamp keeps all-zero
                columns (pad images) finite through the iteration."""
                pairs = PGROUPS[g]
                lo, hi = pairs[0] * 2 * CPC, (pairs[-1] + 1) * 2 * CPC
                n = hi - lo

                def tl(tag, dt=f32):
                    t = nwt.tile([PPART, NMAX], dt, tag=tag, name=f"nwt_{tag}")
                    return t[:, :n]

                x = tl("x")
                nc.vector.tensor_scalar_max(x, s2_all[:, lo:hi], 1e-20)
                t1 = tl("t1", i32)
                nc.vector.tensor_scalar(
                    t1, x.bitcast(i32), 1, None, op0=ALU.logical_shift_right
                )
                y0 = tl("y0")
                nc.vector.tensor_tensor(
                    y0.bitcast(i32),
                    magic[:].broadcast_to([PPART, n]),
                    t1,
                    op=ALU.subtract,
                )
                # one Newton step with the 9x softmax temperature folded in:
                # rn9 = (13.5 + (-4.5 * y0^2) * x) * y0 = 9 * rsqrt(x)
                # (seed err ~3.4% -> ~0.2% after one step; loss impact
                # measured at ~4e-5 relative, far inside tolerance)
                a1 = tl("a1")
                nc.vector.tensor_mul(a1, y0, y0)
                nc.vector.scalar_tensor_tensor(a1, a1, -4.5, x, ALU.mult,
                                               ALU.mult)
                nc.vector.scalar_tensor_tensor(rn9_all[:, lo:hi], a1, 13.5,
                                               y0, ALU.add, ALU.mult)

            an_t: dict = {}

            def sweep2_an(j):
                # an = al * (9 * rsqrt(s2)) broadcast over words  (GPSIMD,
                # one op per slot-width region covering both packs)
                al = al_t.pop(j)
                an = anpool.tile([PPART, 2 * NCW], f32, tag="an")
                anr = an[:].rearrange("p (u x) -> p u x", u=2)
                alr = al[:].rearrange("p (u x) -> p u x", u=2)
                rn = rn9_all[:, j * 2 * CPC:(j + 1) * 2 * CPC].rearrange(
                    "p (u c) -> p u c", u=2)
                if ns > 0:
                    nc.gpsimd.tensor_mul(
                        anr[:, :, 0:SLB].rearrange("p u (c w) -> p u c w",
                                                   c=ns),
                        alr[:, :, 0:SLB].rearrange("p u (c w) -> p u c w",
                                                   c=ns),
                        rn[:, :, 0:ns].broadcast_to([PPART, 2, ns, ws]),
                    )
                if nl > 0:
                    nc.gpsimd.tensor_mul(
                        anr[:, :, SLB:NCW].rearrange("p u (c w) -> p u c w",
                                                     c=nl),
                        alr[:, :, SLB:NCW].rearrange("p u (c w) -> p u c w",
                                                     c=nl),
                        rn[:, :, ns:CPC].broadcast_to([PPART, 2, nl, W]),
                    )
                an_t[j] = an

            def sweep2_exp(j):
                an = an_t.pop(j)
                e = epool.tile([PPART, 2 * NCW], bf16, tag="e")
                nc.scalar.activation(e[:], an[:], AF.Exp)
                e_t[j] = e

            def sweep2_rest(j):
                a_sb, pk_sb, e = asb_t.pop(j), pk_t.pop(j), e_t.pop(j)

                # H = Gbd @ E per pack; ea/eh pair-wide on DVE; the two
                # r-reduction accumulations (G/ones padded to 128 for FWL)
                h_ps = psH.tile([128, 2 * PB], f32)
                for u in range(2):
                    nc.tensor.matmul(
                        h_ps[:, u * PB:u * PB + NCW],
                        pk_sb[:PPART, u * PKC + IMC:u * PKC + IMC + SW],
                        e[:, u * NCW:(u + 1) * NCW],
                        start=True, stop=True,
                    )
                ea = eapool.tile([PPART, 2 * NCW], bf16, tag="ea")
                nc.vector.tensor_mul(ea[:], e[:], a_sb[:])
                for u in range(2):
                    p = 2 * j + u
                    nc.tensor.matmul(
                        nz_acc[:],
                        pk_sb[:PPART, u * PKC + IMC + SW:u * PKC + IMC + 2 * SW],
                        ea[:, u * NCW:(u + 1) * NCW],
                        start=(p == 0), stop=(p == NPACK - 1),
                    )
                eh = eapool.tile([PPART, 2 * NCW], bf16, tag="eh")
                nc.vector.tensor_mul(
                    eh[:].rearrange("p (u x) -> p u x", u=2),
                    e[:].rearrange("p (u x) -> p u x", u=2),
                    pair_view(h_ps),
                )
                for u in range(2):
                    p = 2 * j + u
                    nc.tensor.matmul(
                        wz_acc[:],
                        pk_sb[:PPART, u * PKC + IMC + SW:u * PKC + IMC + 2 * SW],
                        eh[:, u * NCW:(u + 1) * NCW],
                        start=(p == 0), stop=(p == NPACK - 1),
                    )

            # ---- software-pipelined emission over pair steps ----
            # Per step t the per-engine queue order is arranged so nothing
            # blocks in-order: GPSIMD computes an(j+1) a step early, ACT
            # runs Exp(j) before sweep1(t)'s Prelu/Copy/Square, and the PE
            # queue holds [A-burst(t), H/acc(j)] so e(j) is long done when
            # PE reaches H(j).
            newton_done = [False] * len(PGROUPS)
            an_step: dict = {}
            n_an = 0     # next pair needing an
            n_rest = 0   # next pair needing e/H/ea/eh/accs
            t = 0
            while n_rest < NPAIR:
                if (n_an < NPAIR and n_an < t
                        and newton_done[group_of[n_an]]):
                    sweep2_an(n_an)
                    an_step[n_an] = t
                    n_an += 1
                lag = 2 if t < NPAIR else 0
                do_rest = (n_rest < n_an
                           and an_step[n_rest] <= t - lag)
                if do_rest:
                    sweep2_exp(n_rest)
                if t < NPAIR:
                    sweep1_mm(t)
                if do_rest:
                    sweep2_rest(n_rest)
                    n_rest += 1
                if t < NPAIR:
                    sweep1_post(t)
                    g = group_of[t]
                    if t == PGROUPS[g][-1]:
                        newton(g)
                        newton_done[g] = True
                t += 1

            # ---- phase 2: sim = ns * rsqrt(ws) / cn, LSE over words ----
            # rsqrt on DVE (magic + 1 Newton step) so no ACT table switch
            # is needed: the exp table from the main loop stays loaded.
            wsm = ph2.tile([BP, NCW], f32, tag="wsm")
            nc.vector.tensor_scalar_max(wsm[:], wz_acc[0:BP, :], 1e-20)
            pt1 = ph2.tile([BP, NCW], i32, tag="pt1")
            nc.vector.tensor_scalar(
                pt1[:], wsm[:].bitcast(i32), 1, None,
                op0=ALU.logical_shift_right
            )
            py0 = ph2.tile([BP, NCW], f32, tag="py0")
            nc.vector.tensor_tensor(
                py0[:].bitcast(i32),
                magic[0:BP].broadcast_to([BP, NCW]),
                pt1[:],
                op=ALU.subtract,
            )
            pa1 = ph2.tile([BP, NCW], f32, tag="pa1")
            nc.vector.tensor_mul(pa1[:], py0[:], py0[:])
            nc.vector.scalar_tensor_tensor(pa1[:], pa1[:], -0.5, wsm[:],
                                           ALU.mult, ALU.mult)
            rn = ph2.tile([BP, NCW], f32, tag="rn")
            nc.vector.scalar_tensor_tensor(rn[:], pa1[:], 1.5, py0[:],
                                           ALU.add, ALU.mult)
            # sim = (rsqrt(ws) * (1/cn)) * ns
            qq = ph2.tile([BP, NCW], f32, tag="qq")
            nc.vector.tensor_mul(qq[:], rn[:], nlc_sb[:])
            sim = ph2.tile([BP, NCW], f32, tag="sim")
            nc.vector.tensor_mul(sim[:], qq[:], nz_acc[0:BP, :])
            ee = ph2.tile([BP, NCW], f32, tag="ee")
            nc.scalar.activation(ee[:], sim[:], AF.Exp, scale=LAM_LSE)
            rowz = ph2.tile([BP, CPC], f32, tag="rowz")
            eer = ee[:]
            if ns > 0:
                nc.vector.tensor_reduce(
                    rowz[:, 0:ns],
                    eer[:, 0:SLB].rearrange("p (c w) -> p c w", c=ns),
                    AX.X, ALU.add,
                )
            if nl > 0:
                nc.vector.tensor_reduce(
                    rowz[:, ns:CPC],
                    eer[:, SLB:NCW].rearrange("p (c w) -> p c w", c=nl),
                    AX.X, ALU.add,
                )
            rowc = ph2.tile([BP, CPC], f32, tag="rowc")
            nc.vector.tensor_sub(rowc[:], rowz[:], pc_sb[:])

            # host finishes with ln(rowc)/6
            nc.sync.dma_start(out_d[:], rowc[0:B, :])

    nc.compile()
    return nc


def prepare_inputs(im: np.ndarray, s: np.ndarray, s_l: np.ndarray):
    """Host-side input marshalling: shard captions, transpose to d-major,
    pack images into 3-image/108-partition packs (paired, one DMA buffer
    per 2 packs), block-diagonal Gram + ones stationaries, -ln(caption
    norms) and pad counts."""
    import ml_dtypes

    bf16 = ml_dtypes.bfloat16
    im = np.ascontiguousarray(np.asarray(im, np.float32))
    s = np.ascontiguousarray(np.asarray(s, np.float32))
    s_l = np.asarray(s_l).astype(np.int64)

    # zero out padded words so A columns for padded (c, w) are exactly 0
    wmask = (np.arange(W)[None, :] < s_l[:, None])          # [64, 50]
    s_z = s * wmask[:, :, None].astype(np.float32)

    # im packs: [22, 128, 8*128], each 128-col chunk = 108 real + 20 zero
    imf = im.transpose(2, 0, 1).reshape(D, B * R)            # [1024, 2304]
    imf66 = np.zeros((D, BP * R), np.float32)
    imf66[:, : B * R] = imf
    im108 = (
        imf66.reshape(KCH, 128, NPACK, PPART)
        .transpose(2, 1, 0, 3)
        .reshape(NPACK, 128, KCH, PPART)
    )
    im_packed = np.zeros((NPACK, 128, KCH, SW), np.float32)
    im_packed[:, :, :, :PPART] = im108
    im_packed = im_packed.reshape(NPACK, 128, IMC)

    # Gram matrices, block-diagonal per pack: [22, 108, 108]
    G = np.matmul(im, im.transpose(0, 2, 1))                 # [64, 36, 36] f32
    gbd = np.zeros((NPACK, PPART, PPART), np.float32)
    for jj in range(PACK):
        for p in range(NPACK):
            b = PACK * p + jj
            if b < B:
                gbd[p, R * jj: R * (jj + 1), R * jj: R * (jj + 1)] = G[b]

    # ones_p stationaries: [22, 108, 66], 3 block-ones columns at 3p
    ones_p = np.zeros((NPACK, PPART, BP), np.float32)
    for p in range(NPACK):
        for jj in range(PACK):
            ones_p[p, R * jj: R * (jj + 1), PACK * p + jj] = 1.0

    # fused per-pack buffer [22, 128, 1024 im | 128 G | 128 ones], paired
    pkb = np.zeros((NPACK, 128, PKC), np.float32)
    pkb[:, :, :IMC] = im_packed
    pkb[:, :PPART, IMC:IMC + PPART] = gbd
    pkb[:, :PPART, IMC + SW:IMC + SW + BP] = ones_p
    pk = np.ascontiguousarray(
        pkb.reshape(NPAIR, 2, 128, PKC).transpose(0, 2, 1, 3)
        .reshape(NPAIR, 128, 2 * PKC).astype(bf16)
    )

    # 1/(caption word norms); padded words -> 0 (their ns column is 0,
    # so sim = 0 and each contributes exp(0)=1 to the row sum)
    cn = np.sqrt((s_z * s_z).sum(axis=2))                    # [64, 50]
    nlc = np.where(cn > 0, 1.0 / np.maximum(cn, 1e-30), 0.0).astype(
        np.float32
    )
    padc = (W - s_l).astype(np.float32)                      # [64]

    # slot widths per the two-width layout
    ws, ns, caps = choose_layout(s_l)
    widths = [ws] * ns + [W] * (CPC - ns)
    ncw = sum(widths)

    in_maps = []
    for c in range(NCORES):
        cc = caps[c]                                          # slot -> caption
        s_cols = np.concatenate(
            [s_z[cc[k], :widths[k], :] for k in range(CPC)], axis=0
        )                                                     # [ncw, 1024]
        sT = s_cols.T                                         # [1024, ncw]
        s_packed = np.ascontiguousarray(
            sT.reshape(KCH, 128, ncw).transpose(1, 0, 2).reshape(128, KCH * ncw)
            .astype(bf16)
        )
        nlc_c = np.concatenate([nlc[cc[k], :widths[k]] for k in range(CPC)])
        padc_c = np.array(
            [widths[k] - s_l[cc[k]] for k in range(CPC)], np.float32
        )
        nlc66 = np.broadcast_to(nlc_c.reshape(1, ncw), (BP, ncw))
        padc66 = np.broadcast_to(padc_c.reshape(1, CPC), (BP, CPC))
        in_maps.append(
            {
                "pk": pk,
                "s_packed": s_packed,
                "neglncn": np.ascontiguousarray(nlc66, dtype=np.float32),
                "padc66": np.ascontiguousarray(padc66, dtype=np.float32),
            }
        )
    return in_maps


def scores_from_results(res, s_l) -> np.ndarray:
    """res: list of per-core result dicts -> full [64, 64] score matrix,
    un-permuting the two-width slot assignment."""
    _, _, caps = choose_layout(s_l)
    scores = np.empty((B, B), np.float32)
    for c in range(NCORES):
        rowc = np.asarray(res[c]["scores8"], np.float32)      # [64, 8]
        sc = np.log(np.maximum(rowc, 1e-30)) / LAM_LSE
        for k in range(CPC):
            scores[:, caps[c, k]] = sc[:, k]
    return scores


def margin_loss(scores: np.ndarray) -> np.float32:
    scores = scores.astype(np.float32)
    diag = np.diag(scores).copy()
    cost_s = np.maximum(MARGIN + scores - diag[:, None], 0.0)
    cost_im = np.maximum(MARGIN + scores - diag[None, :], 0.0)
    np.fill_diagonal(cost_s, 0.0)
    np.fill_diagonal(cost_im, 0.0)
    return np.float32(cost_s.max(axis=1).sum() + cost_im.max(axis=0).sum())


def kernel(im: np.ndarray, s: np.ndarray, s_l: np.ndarray) -> np.ndarray:
    from concourse.bass_utils import run_bass_kernel_spmd

    ws, ns, _ = choose_layout(s_l)
    key = (ws, ns)
    if key not in _PROGRAM_CACHE:
        _PROGRAM_CACHE[key] = build_program(ws, ns)
    nc = _PROGRAM_CACHE[key]

    in_maps = prepare_inputs(im, s, s_l)
    res = run_bass_kernel_spmd(nc, in_maps, list(range(NCORES))).results
    return margin_loss(scores_from_results(res, s_l))
